# revision 38
# baseline (speedup 1.0000x reference)
"""Multi-head attention forward on 8 Trainium2 NeuronCores.

Problem: x [2,2048,1024], weights wq/wk/wv/wo [1024,1024] (torch Linear
layout, y = x @ W.T), 16 heads, head_dim 64, fp32.

Sharding: core c handles batch b = c//4 and head group g = c%4 (heads
4g..4g+3, i.e. 256 output dims of wq/wk/wv and 256 input dims of wo).
Each core computes a partial output [2048, 1024]; the host sums the 4
partials per batch (the reduce is host-side, no collectives).

On-core plan:
  Projections in float32r (fp32 data, PE rounds to ~13-bit mantissa,
  1 cycle/row at free-dim >= 512); attention operands (qT/kT/v/pT) in
  bf16 (1-cycle LDWEIGHTS + FWL, ~halves the PE weight-load overhead).
  qT, kT [256, 2048] = W_g @ x^T          (x^T supplied by host)
  v      [2048, 256] = x @ W_g^T, stored per s-tile with a ones column
                       appended per head (v_aug [128, 65] tiles)
  Attention runs per HEAD-PAIR with the two heads' j-streams inter-
  leaved and scores issued DEPTH=6 steps ahead of the AV matmuls, so
  the PE never stalls on the exp stage (stalls re-arm the HAM clock
  throttle, halving the PE clock):
    per j-tile: scoresT [128 j, 1024 i] = kT_j^T @ qT   (bf16)
        pT = exp(0.125 * scoresT): 12/16 tiles on ACT, 4/16 offloaded
        to the DVE via a 3-pass Schraudolph + quadratic-correction
        custom op (keeps ACT below the PE rate)
        o_aug [65, 1024] += v_aug_j^T @ pT      (row 64 = colsum)
    o_cp = copy(o_aug) to SBUF (frees the PSUM slot for the next pair)
    colsum broadcast across partitions via PE outer product; o_sb[h] =
    o_cp[0:64] * reciprocal_approx_fast(bcast)
  out[i-tile, :] = sum_h o_sb[h]^T @ woT_h (PSUM-accumulated, f32r);
  i-block 0's projection and half of i-block 1's run interleaved inside
  the later attention pairs as PE filler work.
"""

import numpy as np
from contextlib import ExitStack

import concourse.bacc as bacc
import concourse.bass as bass
import concourse.mybir as mybir
import concourse.tile as tile
from concourse.bass_utils import run_bass_kernel_spmd

f32 = mybir.dt.float32
f32r = mybir.dt.float32r
bf16 = mybir.dt.bfloat16
i32 = mybir.dt.int32
EXP = mybir.ActivationFunctionType.Exp

# ---- custom DVE op: exp correction multiply --------------------------------
# Schraudolph-style exp on DVE (3 passes, offloads part of softmax's exp from
# the ACT engine, which is the attention-phase bottleneck):
#   p1 (std):  u = int32(score * A + B)     A = 0.125*log2(e)*2^23, B = 127*2^23
#              => bitcast(u) = S = 2^i*(1+f) with i+f = score*0.125*log2(e)
#   p2 (std):  r = (u & 0x7FFFFF) | 0x3F800000        => r = 1+f in [1,2)
#   p3 (cust): out = S * (q0 + r*(q1 + r*q2))  ~= S * 2^f/(1+f) = exp(score/8)
# Correction quadratic fit minimax on [1,2]: rel err <= 6.6e-3, unbiased.
EXP_A = float(0.125 * np.log2(np.e) * 2**23)
EXP_B = float(127 * 2**23)
EXP_MASK = 0x007FFFFF
EXP_OR = 0x3F800000
EXP_Q0 = 1.43400066
EXP_Q1 = -0.66623009
EXP_Q2 = 0.22566318

_EXP_CORR = None


def _ensure_exp_corr():
    global _EXP_CORR
    if _EXP_CORR is not None:
        return _EXP_CORR
    import concourse.dve_ops as dve_ops
    from concourse.dve_spec import Spec, Src0, Src1, C0, C1, C2

    def _ref(in0, in1, c0, c1, c2):
        return in1 * (c2 + in0 * (c0 + in0 * c1))

    op = dve_ops.DveOp(
        "EXP_CORR_ANT",
        Spec(body=Src1 * (C2 + Src0 * (C0 + Src0 * C1)), reference=_ref),
        subdim=False,
        uops_sha={},
    )
    if op.name not in dve_ops._SUB_OPCODE_FOR_NAME:
        dve_ops.OPS.append(op)
        dve_ops.CUSTOM_DVE_SPECS[op.name] = op.spec
        dve_ops._SUB_OPCODE_FOR_NAME[op.name] = (
            max(dve_ops._SUB_OPCODE_FOR_NAME.values()) + 1
        )
    # pin the uops sha (first compile reports the computed value)
    from concourse.bass_utils import dve_ver_for

    for ver in ("v3",):
        try:
            op.compile(ver)
        except ValueError as e:
            msg = str(e)
            got = msg.split(f"{ver}: ")[1].split(" ")[0]
            op.uops_sha[ver] = got
            op.compile(ver)
    _EXP_CORR = op
    return op

B, S, D = 2, 2048, 1024
H, DH = 16, 64
NCORES = 8
GROUPS = NCORES // B           # 4 head-groups per batch
HPC = H // GROUPS              # 4 heads per core
DLOC = HPC * DH                # 256
KT = D // 128                  # 8 contraction tiles
ST = S // 128                  # 16 sequence tiles
NB = 2                         # i-blocks
IB = S // NB                   # 1024
NCH = IB // 512                # 512-wide matmul chunks per i-block


def _emit(tc, nc):
    xT = nc.dram_tensor("xT", [D, S], f32, kind="ExternalInput").ap()
    wqT = nc.dram_tensor("wqT", [D, DLOC], f32, kind="ExternalInput").ap()
    wkT = nc.dram_tensor("wkT", [D, DLOC], f32, kind="ExternalInput").ap()
    wvT = nc.dram_tensor("wvT", [D, DLOC], f32, kind="ExternalInput").ap()
    woT = nc.dram_tensor("woT", [DLOC, D], f32, kind="ExternalInput").ap()
    outp = nc.dram_tensor("outp", [S, D], f32, kind="ExternalOutput").ap()

    with ExitStack() as ctx:
        wpool = ctx.enter_context(tc.tile_pool(name="wpool", bufs=1))
        qkv = ctx.enter_context(tc.tile_pool(name="qkv", bufs=1))
        small = ctx.enter_context(tc.tile_pool(name="small", bufs=2))
        ps = ctx.enter_context(tc.tile_pool(name="ps", bufs=2, space="PSUM"))
        pso = ctx.enter_context(tc.tile_pool(name="pso", bufs=2, space="PSUM"))

        # ---- constants ----
        ones_f = small.tile([128, HPC], f32, bufs=1)
        nc.vector.memset(ones_f, 1.0)
        ones65f = small.tile([65, 64], f32, bufs=1)
        nc.vector.memset(ones65f, 1.0)
        ones65 = small.tile([65, 64], f32r, bufs=1)
        nc.vector.tensor_copy(ones65, ones65f)

        # ---- load weights (ACT HWDGE queue) and x^T (SP HWDGE queue) as
        # f32, cast to f32r on the idle DVE ----
        with tc.tile_pool(name="xtpool", bufs=1) as xtpool:
            wts = []
            for name, src in (("wq", wqT), ("wk", wkT), ("wv", wvT)):
                w_r = xtpool.tile([128, KT, DLOC], f32r, name=f"{name}_r", tag=name)
                srcv = src.rearrange("(k p) m -> p k m", p=128)
                for k in range(KT):
                    nc.gpsimd.dma_start(out=w_r[:, k], in_=srcv[:, k])
                wts.append(w_r)
            wq_r, wk_r, wv_r = wts

            # woT [256, 1024] -> [64 part, HPC, 1024] (head on free axis)
            wo_r = wpool.tile([64, HPC, D], f32r)
            wov = woT.rearrange("(h c) e -> c h e", c=64)
            for h in range(HPC):
                nc.gpsimd.dma_start(out=wo_r[:, h], in_=wov[:, h])

            # x^T via fast HWDGE + DVE cast (DVE is idle during the load)
            xt_r = xtpool.tile([128, KT, S], f32r)
            xv = xT.rearrange("(k p) s -> p k s", p=128)
            with tc.tile_pool(name="stage", bufs=4) as stage:
                for k in range(KT):
                    st_t = stage.tile([128, 2048], f32, tag="stage", name="st_x")
                    eng = nc.sync if k % 2 == 0 else nc.scalar
                    eng.dma_start(out=st_t, in_=xv[:, k])
                    nc.vector.tensor_copy(xt_r[:, k], st_t)

            # ---- projections ----
            # v [s, dloc] with ones column: v_sb [128, st, h, 65]
            v_sb = qkv.tile([128, ST, HPC, 65], bf16)
            for st_i in range(ST):
                pv = ps.tile([128, DLOC], f32, tag="ps", name="pv")
                for k in range(KT):
                    nc.tensor.matmul(
                        pv,
                        lhsT=xt_r[:, k, st_i * 128 : (st_i + 1) * 128],
                        rhs=wv_r[:, k],
                        start=(k == 0),
                        stop=(k == KT - 1),
                    )
                nc.vector.tensor_copy(
                    v_sb[:, st_i, :, 0:64],
                    pv.rearrange("p (h d) -> p h d", h=HPC),
                )
                nc.vector.tensor_copy(v_sb[:, st_i, :, 64], ones_f)

            # qT, kT [128, 2 m-tiles, S]
            qt = qkv.tile([128, 2, S], bf16)
            kt = qkv.tile([128, 2, S], bf16)
            for dst, w_r in ((qt, wq_r), (kt, wk_r)):
                for m in range(2):
                    for half in range(2):
                        pq = ps.tile([128, IB], f32, tag="ps", name="pq")
                        for k in range(KT):
                            for ch in range(NCH):
                                nc.tensor.matmul(
                                    pq[:, ch * 512 : (ch + 1) * 512],
                                    lhsT=w_r[:, k, m * 128 : (m + 1) * 128],
                                    rhs=xt_r[
                                        :,
                                        k,
                                        half * IB
                                        + ch * 512 : half * IB
                                        + (ch + 1) * 512,
                                    ],
                                    start=(k == 0),
                                    stop=(k == KT - 1),
                                )
                        nc.vector.tensor_copy(
                            dst[:, m, half * IB : (half + 1) * IB], pq
                        )

        # ---- phase C/D pools (allocated after staging space is released) ----
        ptp = ctx.enter_context(tc.tile_pool(name="ptp", bufs=8))
        osb = ctx.enter_context(tc.tile_pool(name="osb", bufs=1))
        outsb = ctx.enter_context(tc.tile_pool(name="outsb", bufs=3))
        norm = ctx.enter_context(tc.tile_pool(name="norm", bufs=2))
        # o^T accumulator in SBUF for all heads/i-blocks (read by phase D)
        o_sb = osb.tile([64, HPC, NB, IB], f32r, name="o_sb")

        exp_corr = _ensure_exp_corr()
        alu = bass.mybir.AluOpType
        DVE_JTS = frozenset((2, 5, 8, 11, 14))  # 5/16 exps offloaded to DVE

        def emit_head_pair(ib, h0, extra=None):
            """Attention for heads (h0, h0+1): the two heads' j-streams are
            interleaved so the PE always has the other head's matmuls while an
            exp is in flight. `extra` is a list of closures emitting filler PE
            work (deferred output-projection blocks), spread over the loop."""
            heads = (h0, h0 + 1)
            o_augs = {}
            for h in heads:
                o_augs[h] = pso.tile([65, IB], f32, tag="pso", name="o_aug")

            def scores(h, jt):
                p0 = (h % 2) * 64
                mi = h // 2
                ssc = ps.tile([128, IB], f32, tag="ps", name="ssc")
                for ch in range(NCH):
                    nc.tensor.matmul(
                        ssc[:, ch * 512 : (ch + 1) * 512],
                        lhsT=kt[p0 : p0 + 64, mi, jt * 128 : (jt + 1) * 128],
                        rhs=qt[
                            p0 : p0 + 64,
                            mi,
                            ib * IB + ch * 512 : ib * IB + (ch + 1) * 512,
                        ],
                        start=True,
                        stop=True,
                    )
                pt = ptp.tile([128, IB], bf16, tag="pt", name="pt")
                if jt in DVE_JTS:
                    ue = ptp.tile([128, IB], i32, tag="ue", name="ue", bufs=4)
                    nc.vector.tensor_scalar(
                        ue, ssc, EXP_A, EXP_B, alu.mult, alu.add
                    )
                    re = ptp.tile([128, IB], i32, tag="re", name="re", bufs=4)
                    nc.vector.tensor_scalar(
                        re, ue, EXP_MASK, EXP_OR, alu.bitwise_and, alu.bitwise_or
                    )
                    nc.vector._custom_dve(
                        exp_corr,
                        out=pt,
                        in0=re.bitcast(f32),
                        in1=ue.bitcast(f32),
                        s0=EXP_Q1,
                        s1=EXP_Q2,
                        imm2=EXP_Q0,
                    )
                else:
                    nc.scalar.activation(pt, ssc, EXP, scale=0.125)
                return pt

            def av(h, jt, pt):
                for ch in range(NCH):
                    nc.tensor.matmul(
                        o_augs[h][:, ch * 512 : (ch + 1) * 512],
                        lhsT=v_sb[:, jt, h, :],
                        rhs=pt[:, ch * 512 : (ch + 1) * 512],
                        start=(jt == 0),
                        stop=(jt == ST - 1),
                    )

            order = [(h, jt) for jt in range(ST) for h in heads]
            DEPTH = 6
            extra = list(extra or [])
            pts = {}
            for n, (h, jt) in enumerate(order):
                pts[(h, jt)] = scores(h, jt)
                if n >= DEPTH:
                    key = order[n - DEPTH]
                    av(*key, pts.pop(key))
                if extra and n % 3 == 2:
                    extra.pop(0)()
            for key in order[-DEPTH:]:
                av(*key, pts.pop(key))
            for fn in extra:
                fn()

            # normalize both heads. First copy o_aug to SBUF (releases the
            # PSUM slot for the next pair immediately), then broadcast the
            # colsum via PE outer product and multiply by its reciprocal.
            for h in heads:
                o_aug = o_augs[h]
                o_cp = norm.tile([65, IB], f32r, tag="ocp", name="o_cp")
                nc.vector.tensor_copy(o_cp, o_aug)
                cb_ps = ps.tile([64, IB], f32, tag="ps", name="cb_ps")
                for ch in range(NCH):
                    nc.tensor.matmul(
                        cb_ps[:, ch * 512 : (ch + 1) * 512],
                        lhsT=ones65[64:65, :],
                        rhs=o_cp[64:65, ch * 512 : (ch + 1) * 512],
                        start=True,
                        stop=True,
                    )
                rb_f = norm.tile([64, IB], f32, tag="rb_f", name="rb_f")
                nc.vector.reciprocal_approx_fast(rb_f, cb_ps)
                nc.vector.tensor_mul(o_sb[:, h, ib], o_cp[0:64, :], rb_f)

        def emit_out_block(ib, it):
            """Output projection for rows [ib*IB + it*128, +128)."""
            po = ps.tile([128, D], f32, tag="ps", name="po")
            for h in range(HPC):
                for ch in range(2):
                    nc.tensor.matmul(
                        po[:, ch * 512 : (ch + 1) * 512],
                        lhsT=o_sb[:, h, ib, it * 128 : (it + 1) * 128],
                        rhs=wo_r[:, h, ch * 512 : (ch + 1) * 512],
                        start=(h == 0),
                        stop=(h == HPC - 1),
                    )
            ot = outsb.tile([128, D], f32, tag="ot", name="ot")
            nc.vector.tensor_copy(ot, po)
            row = ib * IB + it * 128
            nc.sync.dma_start(out=outp[row : row + 128, :], in_=ot)

        # Partial output projection for i-block 1, heads 0-1 only, into an
        # SBUF accumulator (runs interleaved inside the last attention pair
        # so the tail only has heads 2-3 left).
        dacc = ctx.enter_context(tc.tile_pool(name="dacc", bufs=1))
        d1_acc = dacc.tile([128, 8, D], f32, name="d1_acc")

        def emit_d1_part01(it):
            po = ps.tile([128, D], f32, tag="ps", name="po")
            for h in range(2):
                for ch in range(2):
                    nc.tensor.matmul(
                        po[:, ch * 512 : (ch + 1) * 512],
                        lhsT=o_sb[:, h, 1, it * 128 : (it + 1) * 128],
                        rhs=wo_r[:, h, ch * 512 : (ch + 1) * 512],
                        start=(h == 0),
                        stop=(h == 1),
                    )
            nc.vector.tensor_copy(d1_acc[:, it], po)

        def emit_out_block2(it):
            """Tail: heads 2-3 of i-block 1 plus the accumulated 0-1 half."""
            po = ps.tile([128, D], f32, tag="ps", name="po")
            for h in range(2, HPC):
                for ch in range(2):
                    nc.tensor.matmul(
                        po[:, ch * 512 : (ch + 1) * 512],
                        lhsT=o_sb[:, h, 1, it * 128 : (it + 1) * 128],
                        rhs=wo_r[:, h, ch * 512 : (ch + 1) * 512],
                        start=(h == 2),
                        stop=(h == HPC - 1),
                    )
            ot = outsb.tile([128, D], f32, tag="ot", name="ot")
            nc.vector.tensor_add(ot, po, d1_acc[:, it])
            row = IB + it * 128
            nc.sync.dma_start(out=outp[row : row + 128, :], in_=ot)

        # Attention i-block 0 (two head-pairs), then i-block 1 with i-block
        # 0's output projection (and the first half of i-block 1's) inter-
        # leaved as PE filler, then the short tail.
        emit_head_pair(0, 0)
        emit_head_pair(0, 2)
        emit_head_pair(
            1, 0, extra=[lambda it=it: emit_out_block(0, it) for it in range(8)]
        )
        emit_head_pair(
            1, 2, extra=[lambda it=it: emit_d1_part01(it) for it in range(8)]
        )
        for it in range(8):
            emit_out_block2(it)


_PROGRAM = None


def _program():
    global _PROGRAM
    if _PROGRAM is None:
        nc = bacc.Bacc("TRN2", target_bir_lowering=False, debug=False)
        with tile.TileContext(nc) as tc:
            _emit(tc, nc)
        nc.compile()
        _PROGRAM = nc
    return _PROGRAM


def kernel(x, e, wq, wk, wv, wo, **_unused):
    x = np.asarray(x, dtype=np.float32)
    wq = np.asarray(wq, dtype=np.float32)
    wk = np.asarray(wk, dtype=np.float32)
    wv = np.asarray(wv, dtype=np.float32)
    wo = np.asarray(wo, dtype=np.float32)

    nc = _program()
    in_maps = []
    for c in range(NCORES):
        b, g = divmod(c, GROUPS)
        rows = slice(g * DLOC, (g + 1) * DLOC)
        in_maps.append(
            {
                "xT": np.ascontiguousarray(x[b].T),
                "wqT": np.ascontiguousarray(wq[rows, :].T),
                "wkT": np.ascontiguousarray(wk[rows, :].T),
                "wvT": np.ascontiguousarray(wv[rows, :].T),
                "woT": np.ascontiguousarray(wo[:, rows].T),
            }
        )

    res = run_bass_kernel_spmd(nc, in_maps, list(range(NCORES))).results
    out = np.empty((B, S, D), dtype=np.float32)
    for b in range(B):
        acc = res[b * GROUPS]["outp"].astype(np.float32)
        for g in range(1, GROUPS):
            acc = acc + res[b * GROUPS + g]["outp"]
        out[b] = acc
    return out


# revision 39
# speedup vs baseline: 1.0687x; 1.0687x over previous
"""Multi-head attention forward on 8 Trainium2 NeuronCores.

Problem: x [2,2048,1024], weights wq/wk/wv/wo [1024,1024] (torch Linear
layout, y = x @ W.T), 16 heads, head_dim 64, fp32.

Sharding: core c handles batch b = c//4 and head group g = c%4 (heads
4g..4g+3, i.e. 256 output dims of wq/wk/wv and 256 input dims of wo).
Each core computes a partial output [2048, 1024]; the host sums the 4
partials per batch (the reduce is host-side, no collectives).

On-core plan:
  Projections in float32r (fp32 data, PE rounds to ~13-bit mantissa,
  1 cycle/row at free-dim >= 512); attention operands (qT/kT/v/pT) in
  bf16 (1-cycle LDWEIGHTS + FWL, ~halves the PE weight-load overhead).
  qT, kT [256, 2048] = W_g @ x^T          (x^T supplied by host)
  v      [2048, 256] = x @ W_g^T, stored per s-tile with a ones column
                       appended per head (v_aug [128, 65] tiles)
  Attention runs per HEAD-PAIR with the two heads' j-streams inter-
  leaved and scores issued DEPTH=6 steps ahead of the AV matmuls, so
  the PE never stalls on the exp stage (stalls re-arm the HAM clock
  throttle, halving the PE clock):
    per j-tile: scoresT [128 j, 1024 i] = kT_j^T @ qT   (bf16)
        pT = exp(0.125 * scoresT): 12/16 tiles on ACT, 4/16 offloaded
        to the DVE via a 3-pass Schraudolph + quadratic-correction
        custom op (keeps ACT below the PE rate)
        o_aug [65, 1024] += v_aug_j^T @ pT      (row 64 = colsum)
    o_cp = copy(o_aug) to SBUF (frees the PSUM slot for the next pair)
    colsum broadcast across partitions via PE outer product; o_sb[h] =
    o_cp[0:64] * reciprocal_approx_fast(bcast)
  out[i-tile, :] = sum_h o_sb[h]^T @ woT_h (PSUM-accumulated, f32r);
  i-block 0's projection and half of i-block 1's run interleaved inside
  the later attention pairs as PE filler work.
"""

import numpy as np
from contextlib import ExitStack

import concourse.bacc as bacc
import concourse.bass as bass
import concourse.mybir as mybir
import concourse.tile as tile
from concourse.bass_utils import run_bass_kernel_spmd

f32 = mybir.dt.float32
f32r = mybir.dt.float32r
bf16 = mybir.dt.bfloat16
i32 = mybir.dt.int32
EXP = mybir.ActivationFunctionType.Exp

# ---- custom DVE op: exp correction multiply --------------------------------
# Schraudolph-style exp on DVE (3 passes, offloads part of softmax's exp from
# the ACT engine, which is the attention-phase bottleneck):
#   p1 (std):  u = int32(score * A + B)     A = 0.125*log2(e)*2^23, B = 127*2^23
#              => bitcast(u) = S = 2^i*(1+f) with i+f = score*0.125*log2(e)
#   p2 (std):  r = (u & 0x7FFFFF) | 0x3F800000        => r = 1+f in [1,2)
#   p3 (cust): out = S * (q0 + r*(q1 + r*q2))  ~= S * 2^f/(1+f) = exp(score/8)
# Correction quadratic fit minimax on [1,2]: rel err <= 6.6e-3, unbiased.
EXP_A = float(0.125 * np.log2(np.e) * 2**23)
EXP_B = float(127 * 2**23)
EXP_MASK = 0x007FFFFF
EXP_OR = 0x3F800000
EXP_Q0 = 1.43400066
EXP_Q1 = -0.66623009
EXP_Q2 = 0.22566318

_EXP_CORR = None


def _ensure_exp_corr():
    global _EXP_CORR
    if _EXP_CORR is not None:
        return _EXP_CORR
    import concourse.dve_ops as dve_ops
    from concourse.dve_spec import Spec, Src0, Src1, C0, C1, C2

    def _ref(in0, in1, c0, c1, c2):
        return in1 * (c2 + in0 * (c0 + in0 * c1))

    op = dve_ops.DveOp(
        "EXP_CORR_ANT",
        Spec(body=Src1 * (C2 + Src0 * (C0 + Src0 * C1)), reference=_ref),
        subdim=False,
        uops_sha={},
    )
    if op.name not in dve_ops._SUB_OPCODE_FOR_NAME:
        dve_ops.OPS.append(op)
        dve_ops.CUSTOM_DVE_SPECS[op.name] = op.spec
        dve_ops._SUB_OPCODE_FOR_NAME[op.name] = (
            max(dve_ops._SUB_OPCODE_FOR_NAME.values()) + 1
        )
    # pin the uops sha (first compile reports the computed value)
    from concourse.bass_utils import dve_ver_for

    for ver in ("v3",):
        try:
            op.compile(ver)
        except ValueError as e:
            msg = str(e)
            got = msg.split(f"{ver}: ")[1].split(" ")[0]
            op.uops_sha[ver] = got
            op.compile(ver)
    _EXP_CORR = op
    return op

B, S, D = 2, 2048, 1024
H, DH = 16, 64
NCORES = 8
GROUPS = NCORES // B           # 4 head-groups per batch
HPC = H // GROUPS              # 4 heads per core
DLOC = HPC * DH                # 256
KT = D // 128                  # 8 contraction tiles
ST = S // 128                  # 16 sequence tiles
NB = 2                         # i-blocks
IB = S // NB                   # 1024
NCH = IB // 512                # 512-wide matmul chunks per i-block


def _emit(tc, nc):
    xT = nc.dram_tensor("xT", [D, S], f32, kind="ExternalInput").ap()
    wqT = nc.dram_tensor("wqT", [D, DLOC], f32, kind="ExternalInput").ap()
    wkT = nc.dram_tensor("wkT", [D, DLOC], f32, kind="ExternalInput").ap()
    wvT = nc.dram_tensor("wvT", [D, DLOC], f32, kind="ExternalInput").ap()
    woT = nc.dram_tensor("woT", [DLOC, D], f32, kind="ExternalInput").ap()
    outp = nc.dram_tensor("outp", [S, D], f32, kind="ExternalOutput").ap()

    with ExitStack() as ctx:
        wpool = ctx.enter_context(tc.tile_pool(name="wpool", bufs=1))
        qkv = ctx.enter_context(tc.tile_pool(name="qkv", bufs=1))
        small = ctx.enter_context(tc.tile_pool(name="small", bufs=2))
        ps = ctx.enter_context(tc.tile_pool(name="ps", bufs=2, space="PSUM"))
        pso = ctx.enter_context(tc.tile_pool(name="pso", bufs=2, space="PSUM"))

        # ---- constants ----
        ones_f = small.tile([128, HPC], f32, bufs=1)
        nc.vector.memset(ones_f, 1.0)
        ones65f = small.tile([65, 64], f32, bufs=1)
        nc.vector.memset(ones65f, 1.0)
        ones65 = small.tile([65, 64], f32r, bufs=1)
        nc.vector.tensor_copy(ones65, ones65f)

        # ---- load weights (ACT HWDGE queue) and x^T (SP HWDGE queue) as
        # f32, cast to f32r on the idle DVE ----
        with tc.tile_pool(name="xtpool", bufs=1) as xtpool:
            wts = []
            for name, src in (("wq", wqT), ("wk", wkT), ("wv", wvT)):
                w_r = xtpool.tile([128, KT, DLOC], f32r, name=f"{name}_r", tag=name)
                srcv = src.rearrange("(k p) m -> p k m", p=128)
                for k in range(KT):
                    nc.gpsimd.dma_start(out=w_r[:, k], in_=srcv[:, k])
                wts.append(w_r)
            wq_r, wk_r, wv_r = wts

            # woT [256, 1024] -> [64 part, HPC, 1024] (head on free axis)
            wo_r = wpool.tile([64, HPC, D], f32r)
            wov = woT.rearrange("(h c) e -> c h e", c=64)
            for h in range(HPC):
                nc.gpsimd.dma_start(out=wo_r[:, h], in_=wov[:, h])

            # x^T via fast HWDGE + DVE cast (DVE is idle during the load)
            xt_r = xtpool.tile([128, KT, S], f32r)
            xv = xT.rearrange("(k p) s -> p k s", p=128)
            with tc.tile_pool(name="stage", bufs=4) as stage:
                for k in range(KT):
                    st_t = stage.tile([128, 2048], f32, tag="stage", name="st_x")
                    eng = nc.sync if k % 2 == 0 else nc.scalar
                    eng.dma_start(out=st_t, in_=xv[:, k])
                    nc.vector.tensor_copy(xt_r[:, k], st_t)

            # ---- projections ----
            # v [s, dloc] with ones column: v_sb [128, st, h, 65]
            v_sb = qkv.tile([128, ST, HPC, 65], bf16)
            for st_i in range(ST):
                pv = ps.tile([128, DLOC], f32, tag="ps", name="pv")
                for k in range(KT):
                    nc.tensor.matmul(
                        pv,
                        lhsT=xt_r[:, k, st_i * 128 : (st_i + 1) * 128],
                        rhs=wv_r[:, k],
                        start=(k == 0),
                        stop=(k == KT - 1),
                    )
                nc.vector.tensor_copy(
                    v_sb[:, st_i, :, 0:64],
                    pv.rearrange("p (h d) -> p h d", h=HPC),
                )
                nc.vector.tensor_copy(v_sb[:, st_i, :, 64], ones_f)

            # qT, kT [128, 2 m-tiles, S]
            qt = qkv.tile([128, 2, S], bf16)
            kt = qkv.tile([128, 2, S], bf16)
            for dst, w_r in ((qt, wq_r), (kt, wk_r)):
                for m in range(2):
                    for half in range(2):
                        pq = ps.tile([128, IB], f32, tag="ps", name="pq")
                        for k in range(KT):
                            for ch in range(NCH):
                                nc.tensor.matmul(
                                    pq[:, ch * 512 : (ch + 1) * 512],
                                    lhsT=w_r[:, k, m * 128 : (m + 1) * 128],
                                    rhs=xt_r[
                                        :,
                                        k,
                                        half * IB
                                        + ch * 512 : half * IB
                                        + (ch + 1) * 512,
                                    ],
                                    start=(k == 0),
                                    stop=(k == KT - 1),
                                )
                        nc.vector.tensor_copy(
                            dst[:, m, half * IB : (half + 1) * IB], pq
                        )

        # ---- phase C/D pools (allocated after staging space is released) ----
        ptp = ctx.enter_context(tc.tile_pool(name="ptp", bufs=8))
        osb = ctx.enter_context(tc.tile_pool(name="osb", bufs=1))
        outsb = ctx.enter_context(tc.tile_pool(name="outsb", bufs=3))
        norm = ctx.enter_context(tc.tile_pool(name="norm", bufs=2))
        # o^T accumulator in SBUF for all heads/i-blocks (read by phase D)
        o_sb = osb.tile([64, HPC, NB, IB], f32r, name="o_sb")

        exp_corr = _ensure_exp_corr()
        alu = bass.mybir.AluOpType
        DVE_JTS = frozenset((3, 7, 11, 14))  # 4/16 exps offloaded to DVE

        def emit_head_pair(ib, h0, extra=None):
            """Attention for heads (h0, h0+1): the two heads' j-streams are
            interleaved so the PE always has the other head's matmuls while an
            exp is in flight. `extra` is a list of closures emitting filler PE
            work (deferred output-projection blocks), spread over the loop."""
            heads = (h0, h0 + 1)
            o_augs = {}
            for h in heads:
                o_augs[h] = pso.tile([65, IB], f32, tag="pso", name="o_aug")

            def scores(h, jt):
                p0 = (h % 2) * 64
                mi = h // 2
                ssc = ps.tile([128, IB], f32, tag="ps", name="ssc")
                for ch in range(NCH):
                    nc.tensor.matmul(
                        ssc[:, ch * 512 : (ch + 1) * 512],
                        lhsT=kt[p0 : p0 + 64, mi, jt * 128 : (jt + 1) * 128],
                        rhs=qt[
                            p0 : p0 + 64,
                            mi,
                            ib * IB + ch * 512 : ib * IB + (ch + 1) * 512,
                        ],
                        start=True,
                        stop=True,
                    )
                pt = ptp.tile([128, IB], bf16, tag="pt", name="pt")
                if jt in DVE_JTS:
                    ue = ptp.tile([128, IB], i32, tag="ue", name="ue", bufs=4)
                    nc.vector.tensor_scalar(
                        ue, ssc, EXP_A, EXP_B, alu.mult, alu.add
                    )
                    re = ptp.tile([128, IB], i32, tag="re", name="re", bufs=4)
                    nc.vector.tensor_scalar(
                        re, ue, EXP_MASK, EXP_OR, alu.bitwise_and, alu.bitwise_or
                    )
                    nc.vector._custom_dve(
                        exp_corr,
                        out=pt,
                        in0=re.bitcast(f32),
                        in1=ue.bitcast(f32),
                        s0=EXP_Q1,
                        s1=EXP_Q2,
                        imm2=EXP_Q0,
                    )
                else:
                    nc.scalar.activation(pt, ssc, EXP, scale=0.125)
                return pt

            def av(h, jt, pt):
                for ch in range(NCH):
                    nc.tensor.matmul(
                        o_augs[h][:, ch * 512 : (ch + 1) * 512],
                        lhsT=v_sb[:, jt, h, :],
                        rhs=pt[:, ch * 512 : (ch + 1) * 512],
                        start=(jt == 0),
                        stop=(jt == ST - 1),
                    )

            order = [(h, jt) for jt in range(ST) for h in heads]
            DEPTH = 6
            extra = list(extra or [])
            pts = {}
            for n, (h, jt) in enumerate(order):
                pts[(h, jt)] = scores(h, jt)
                if n >= DEPTH:
                    key = order[n - DEPTH]
                    av(*key, pts.pop(key))
                if extra and n % 3 == 2:
                    extra.pop(0)()
            for key in order[-DEPTH:]:
                av(*key, pts.pop(key))
            for fn in extra:
                fn()

            # normalize both heads. First copy o_aug to SBUF (releases the
            # PSUM slot for the next pair immediately), then broadcast the
            # colsum via PE outer product and multiply by its reciprocal.
            for h in heads:
                o_aug = o_augs[h]
                o_cp = norm.tile([65, IB], f32r, tag="ocp", name="o_cp")
                nc.vector.tensor_copy(o_cp, o_aug)
                cb_ps = ps.tile([64, IB], f32, tag="ps", name="cb_ps")
                for ch in range(NCH):
                    nc.tensor.matmul(
                        cb_ps[:, ch * 512 : (ch + 1) * 512],
                        lhsT=ones65[64:65, :],
                        rhs=o_cp[64:65, ch * 512 : (ch + 1) * 512],
                        start=True,
                        stop=True,
                    )
                rb_f = norm.tile([64, IB], f32, tag="rb_f", name="rb_f")
                nc.vector.reciprocal_approx_fast(rb_f, cb_ps)
                nc.vector.tensor_mul(o_sb[:, h, ib], o_cp[0:64, :], rb_f)

        def emit_out_block(ib, it):
            """Output projection for rows [ib*IB + it*128, +128)."""
            po = ps.tile([128, D], f32, tag="ps", name="po")
            for h in range(HPC):
                for ch in range(2):
                    nc.tensor.matmul(
                        po[:, ch * 512 : (ch + 1) * 512],
                        lhsT=o_sb[:, h, ib, it * 128 : (it + 1) * 128],
                        rhs=wo_r[:, h, ch * 512 : (ch + 1) * 512],
                        start=(h == 0),
                        stop=(h == HPC - 1),
                    )
            ot = outsb.tile([128, D], f32, tag="ot", name="ot")
            nc.vector.tensor_copy(ot, po)
            row = ib * IB + it * 128
            nc.sync.dma_start(out=outp[row : row + 128, :], in_=ot)

        # Partial output projection for i-block 1, heads 0-1 only, into an
        # SBUF accumulator (runs interleaved inside the last attention pair
        # so the tail only has heads 2-3 left).
        dacc = ctx.enter_context(tc.tile_pool(name="dacc", bufs=1))
        d1_acc = dacc.tile([128, 8, D], f32, name="d1_acc")

        def emit_d1_part01(it):
            po = ps.tile([128, D], f32, tag="ps", name="po")
            for h in range(2):
                for ch in range(2):
                    nc.tensor.matmul(
                        po[:, ch * 512 : (ch + 1) * 512],
                        lhsT=o_sb[:, h, 1, it * 128 : (it + 1) * 128],
                        rhs=wo_r[:, h, ch * 512 : (ch + 1) * 512],
                        start=(h == 0),
                        stop=(h == 1),
                    )
            nc.vector.tensor_copy(d1_acc[:, it], po)

        def emit_out_block2(it):
            """Tail: heads 2-3 of i-block 1 plus the accumulated 0-1 half."""
            po = ps.tile([128, D], f32, tag="ps", name="po")
            for h in range(2, HPC):
                for ch in range(2):
                    nc.tensor.matmul(
                        po[:, ch * 512 : (ch + 1) * 512],
                        lhsT=o_sb[:, h, 1, it * 128 : (it + 1) * 128],
                        rhs=wo_r[:, h, ch * 512 : (ch + 1) * 512],
                        start=(h == 2),
                        stop=(h == HPC - 1),
                    )
            ot = outsb.tile([128, D], f32, tag="ot", name="ot")
            nc.vector.tensor_add(ot, po, d1_acc[:, it])
            row = IB + it * 128
            nc.sync.dma_start(out=outp[row : row + 128, :], in_=ot)

        # Attention i-block 0 (two head-pairs), then i-block 1 with i-block
        # 0's output projection (and the first half of i-block 1's) inter-
        # leaved as PE filler, then the short tail.
        emit_head_pair(0, 0)
        emit_head_pair(0, 2)
        emit_head_pair(
            1, 0, extra=[lambda it=it: emit_out_block(0, it) for it in range(8)]
        )
        emit_head_pair(
            1, 2, extra=[lambda it=it: emit_d1_part01(it) for it in range(8)]
        )
        for it in range(8):
            emit_out_block2(it)


_PROGRAM = None


def _program():
    global _PROGRAM
    if _PROGRAM is None:
        nc = bacc.Bacc("TRN2", target_bir_lowering=False, debug=False)
        with tile.TileContext(nc) as tc:
            _emit(tc, nc)
        nc.compile()
        _PROGRAM = nc
    return _PROGRAM


def kernel(x, e, wq, wk, wv, wo, **_unused):
    x = np.asarray(x, dtype=np.float32)
    wq = np.asarray(wq, dtype=np.float32)
    wk = np.asarray(wk, dtype=np.float32)
    wv = np.asarray(wv, dtype=np.float32)
    wo = np.asarray(wo, dtype=np.float32)

    nc = _program()
    in_maps = []
    for c in range(NCORES):
        b, g = divmod(c, GROUPS)
        rows = slice(g * DLOC, (g + 1) * DLOC)
        in_maps.append(
            {
                "xT": np.ascontiguousarray(x[b].T),
                "wqT": np.ascontiguousarray(wq[rows, :].T),
                "wkT": np.ascontiguousarray(wk[rows, :].T),
                "wvT": np.ascontiguousarray(wv[rows, :].T),
                "woT": np.ascontiguousarray(wo[:, rows].T),
            }
        )

    res = run_bass_kernel_spmd(nc, in_maps, list(range(NCORES))).results
    out = np.empty((B, S, D), dtype=np.float32)
    for b in range(B):
        acc = res[b * GROUPS]["outp"].astype(np.float32)
        for g in range(1, GROUPS):
            acc = acc + res[b * GROUPS + g]["outp"]
        out[b] = acc
    return out


# revision 40
# speedup vs baseline: 1.0905x; 1.0203x over previous
"""Multi-head attention forward on 8 Trainium2 NeuronCores.

Problem: x [2,2048,1024], weights wq/wk/wv/wo [1024,1024] (torch Linear
layout, y = x @ W.T), 16 heads, head_dim 64, fp32.

Sharding: core c handles batch b = c//4 and head group g = c%4 (heads
4g..4g+3, i.e. 256 output dims of wq/wk/wv and 256 input dims of wo).
Each core computes a partial output [2048, 1024]; the host sums the 4
partials per batch (the reduce is host-side, no collectives).

On-core plan:
  Projections in float32r (fp32 data, PE rounds to ~13-bit mantissa,
  1 cycle/row at free-dim >= 512); attention operands (qT/kT/v/pT) in
  bf16 (1-cycle LDWEIGHTS + FWL, ~halves the PE weight-load overhead).
  qT, kT [256, 2048] = W_g @ x^T          (x^T supplied by host)
  v      [2048, 256] = x @ W_g^T, stored per s-tile with a ones column
                       appended per head (v_aug [128, 65] tiles)
  Attention runs per HEAD-PAIR with the two heads' j-streams inter-
  leaved and scores issued DEPTH=6 steps ahead of the AV matmuls, so
  the PE never stalls on the exp stage (stalls re-arm the HAM clock
  throttle, halving the PE clock):
    per j-tile: scoresT [128 j, 1024 i] = kT_j^T @ qT   (bf16)
        pT = exp(0.125 * scoresT): 12/16 tiles on ACT, 4/16 offloaded
        to the DVE via a 3-pass Schraudolph + quadratic-correction
        custom op (keeps ACT below the PE rate)
        o_aug [65, 1024] += v_aug_j^T @ pT      (row 64 = colsum)
    o_cp = copy(o_aug) to SBUF (frees the PSUM slot for the next pair)
    colsum broadcast across partitions via PE outer product; o_sb[h] =
    o_cp[0:64] * reciprocal_approx_fast(bcast)
  out[i-tile, :] = sum_h o_sb[h]^T @ woT_h (PSUM-accumulated, f32r);
  i-block 0's projection and half of i-block 1's run interleaved inside
  the later attention pairs as PE filler work.
"""

import numpy as np
from contextlib import ExitStack

import concourse.bacc as bacc
import concourse.bass as bass
import concourse.mybir as mybir
import concourse.tile as tile
from concourse.bass_utils import run_bass_kernel_spmd

f32 = mybir.dt.float32
f32r = mybir.dt.float32r
bf16 = mybir.dt.bfloat16
i32 = mybir.dt.int32
EXP = mybir.ActivationFunctionType.Exp

# ---- custom DVE op: exp correction multiply --------------------------------
# Schraudolph-style exp on DVE (3 passes, offloads part of softmax's exp from
# the ACT engine, which is the attention-phase bottleneck):
#   p1 (std):  u = int32(score * A + B)     A = 0.125*log2(e)*2^23, B = 127*2^23
#              => bitcast(u) = S = 2^i*(1+f) with i+f = score*0.125*log2(e)
#   p2 (std):  r = (u & 0x7FFFFF) | 0x3F800000        => r = 1+f in [1,2)
#   p3 (cust): out = S * (q0 + r*(q1 + r*q2))  ~= S * 2^f/(1+f) = exp(score/8)
# Correction quadratic fit minimax on [1,2]: rel err <= 6.6e-3, unbiased.
EXP_A = float(0.125 * np.log2(np.e) * 2**23)
EXP_B = float(127 * 2**23)
EXP_MASK = 0x007FFFFF
EXP_OR = 0x3F800000
EXP_Q0 = 1.43400066
EXP_Q1 = -0.66623009
EXP_Q2 = 0.22566318

_EXP_CORR = None


def _ensure_exp_corr():
    global _EXP_CORR
    if _EXP_CORR is not None:
        return _EXP_CORR
    import concourse.dve_ops as dve_ops
    from concourse.dve_spec import Spec, Src0, Src1, C0, C1, C2

    def _ref(in0, in1, c0, c1, c2):
        return in1 * (c2 + in0 * (c0 + in0 * c1))

    op = dve_ops.DveOp(
        "EXP_CORR_ANT",
        Spec(body=Src1 * (C2 + Src0 * (C0 + Src0 * C1)), reference=_ref),
        subdim=False,
        uops_sha={},
    )
    if op.name not in dve_ops._SUB_OPCODE_FOR_NAME:
        dve_ops.OPS.append(op)
        dve_ops.CUSTOM_DVE_SPECS[op.name] = op.spec
        dve_ops._SUB_OPCODE_FOR_NAME[op.name] = (
            max(dve_ops._SUB_OPCODE_FOR_NAME.values()) + 1
        )
    # pin the uops sha (first compile reports the computed value)
    from concourse.bass_utils import dve_ver_for

    for ver in ("v3",):
        try:
            op.compile(ver)
        except ValueError as e:
            msg = str(e)
            got = msg.split(f"{ver}: ")[1].split(" ")[0]
            op.uops_sha[ver] = got
            op.compile(ver)
    _EXP_CORR = op
    return op

B, S, D = 2, 2048, 1024
H, DH = 16, 64
NCORES = 8
GROUPS = NCORES // B           # 4 head-groups per batch
HPC = H // GROUPS              # 4 heads per core
DLOC = HPC * DH                # 256
KT = D // 128                  # 8 contraction tiles
ST = S // 128                  # 16 sequence tiles
NB = 2                         # i-blocks
IB = S // NB                   # 1024
NCH = IB // 512                # 512-wide matmul chunks per i-block


def _emit(tc, nc):
    xT = nc.dram_tensor("xT", [D, S], f32, kind="ExternalInput").ap()
    wqT = nc.dram_tensor("wqT", [D, DLOC], f32, kind="ExternalInput").ap()
    wkT = nc.dram_tensor("wkT", [D, DLOC], f32, kind="ExternalInput").ap()
    wvT = nc.dram_tensor("wvT", [D, DLOC], f32, kind="ExternalInput").ap()
    woT = nc.dram_tensor("woT", [DLOC, D], f32, kind="ExternalInput").ap()
    outp = nc.dram_tensor("outp", [S, D], f32, kind="ExternalOutput").ap()

    with ExitStack() as ctx:
        wpool = ctx.enter_context(tc.tile_pool(name="wpool", bufs=1))
        qkv = ctx.enter_context(tc.tile_pool(name="qkv", bufs=1))
        small = ctx.enter_context(tc.tile_pool(name="small", bufs=2))
        ps = ctx.enter_context(tc.tile_pool(name="ps", bufs=2, space="PSUM"))
        pso = ctx.enter_context(tc.tile_pool(name="pso", bufs=2, space="PSUM"))

        # ---- constants ----
        ones_f = small.tile([128, HPC], f32, bufs=1)
        nc.vector.memset(ones_f, 1.0)
        ones65f = small.tile([65, 64], f32, bufs=1)
        nc.vector.memset(ones65f, 1.0)
        ones65 = small.tile([65, 64], f32r, bufs=1)
        nc.vector.tensor_copy(ones65, ones65f)

        # ---- load weights (ACT HWDGE queue) and x^T (SP HWDGE queue) as
        # f32, cast to f32r on the idle DVE ----
        with tc.tile_pool(name="xtpool", bufs=1) as xtpool:
            wts = []
            for name, src in (("wq", wqT), ("wk", wkT), ("wv", wvT)):
                w_r = xtpool.tile([128, KT, DLOC], bf16, name=f"{name}_r", tag=name)
                srcv = src.rearrange("(k p) m -> p k m", p=128)
                for k in range(KT):
                    nc.gpsimd.dma_start(out=w_r[:, k], in_=srcv[:, k])
                wts.append(w_r)
            wq_r, wk_r, wv_r = wts

            # woT [256, 1024] -> [64 part, HPC, 1024] (head on free axis)
            wo_r = wpool.tile([64, HPC, D], f32r)
            wov = woT.rearrange("(h c) e -> c h e", c=64)
            for h in range(HPC):
                nc.gpsimd.dma_start(out=wo_r[:, h], in_=wov[:, h])

            # x^T via fast HWDGE + DVE cast (DVE is idle during the load)
            xt_r = xtpool.tile([128, KT, S], bf16)
            xv = xT.rearrange("(k p) s -> p k s", p=128)
            with tc.tile_pool(name="stage", bufs=4) as stage:
                for k in range(KT):
                    st_t = stage.tile([128, 2048], f32, tag="stage", name="st_x")
                    eng = nc.sync if k % 2 == 0 else nc.scalar
                    eng.dma_start(out=st_t, in_=xv[:, k])
                    nc.vector.tensor_copy(xt_r[:, k], st_t)

            # ---- projections ----
            # v [s, dloc] with ones column: v_sb [128, st, h, 65]
            v_sb = qkv.tile([128, ST, HPC, 65], bf16)
            for st_i in range(ST):
                pv = ps.tile([128, DLOC], f32, tag="ps", name="pv")
                for k in range(KT):
                    nc.tensor.matmul(
                        pv,
                        lhsT=xt_r[:, k, st_i * 128 : (st_i + 1) * 128],
                        rhs=wv_r[:, k],
                        start=(k == 0),
                        stop=(k == KT - 1),
                    )
                nc.vector.tensor_copy(
                    v_sb[:, st_i, :, 0:64],
                    pv.rearrange("p (h d) -> p h d", h=HPC),
                )
                nc.vector.tensor_copy(v_sb[:, st_i, :, 64], ones_f)

            # qT, kT [128, 2 m-tiles, S]
            qt = qkv.tile([128, 2, S], bf16)
            kt = qkv.tile([128, 2, S], bf16)
            for dst, w_r in ((qt, wq_r), (kt, wk_r)):
                for m in range(2):
                    for half in range(2):
                        pq = ps.tile([128, IB], f32, tag="ps", name="pq")
                        for k in range(KT):
                            for ch in range(NCH):
                                nc.tensor.matmul(
                                    pq[:, ch * 512 : (ch + 1) * 512],
                                    lhsT=w_r[:, k, m * 128 : (m + 1) * 128],
                                    rhs=xt_r[
                                        :,
                                        k,
                                        half * IB
                                        + ch * 512 : half * IB
                                        + (ch + 1) * 512,
                                    ],
                                    start=(k == 0),
                                    stop=(k == KT - 1),
                                )
                        nc.vector.tensor_copy(
                            dst[:, m, half * IB : (half + 1) * IB], pq
                        )

        # ---- phase C/D pools (allocated after staging space is released) ----
        ptp = ctx.enter_context(tc.tile_pool(name="ptp", bufs=8))
        osb = ctx.enter_context(tc.tile_pool(name="osb", bufs=1))
        outsb = ctx.enter_context(tc.tile_pool(name="outsb", bufs=3))
        norm = ctx.enter_context(tc.tile_pool(name="norm", bufs=2))
        # o^T accumulator in SBUF for all heads/i-blocks (read by phase D)
        o_sb = osb.tile([64, HPC, NB, IB], f32r, name="o_sb")

        exp_corr = _ensure_exp_corr()
        alu = bass.mybir.AluOpType
        DVE_JTS = frozenset((3, 7, 11, 14))  # 4/16 exps offloaded to DVE

        def emit_head_pair(ib, h0, extra=None):
            """Attention for heads (h0, h0+1): the two heads' j-streams are
            interleaved so the PE always has the other head's matmuls while an
            exp is in flight. `extra` is a list of closures emitting filler PE
            work (deferred output-projection blocks), spread over the loop."""
            heads = (h0, h0 + 1)
            o_augs = {}
            for h in heads:
                o_augs[h] = pso.tile([65, IB], f32, tag="pso", name="o_aug")

            def scores(h, jt):
                p0 = (h % 2) * 64
                mi = h // 2
                ssc = ps.tile([128, IB], f32, tag="ps", name="ssc")
                for ch in range(NCH):
                    nc.tensor.matmul(
                        ssc[:, ch * 512 : (ch + 1) * 512],
                        lhsT=kt[p0 : p0 + 64, mi, jt * 128 : (jt + 1) * 128],
                        rhs=qt[
                            p0 : p0 + 64,
                            mi,
                            ib * IB + ch * 512 : ib * IB + (ch + 1) * 512,
                        ],
                        start=True,
                        stop=True,
                    )
                pt = ptp.tile([128, IB], bf16, tag="pt", name="pt")
                if jt in DVE_JTS:
                    ue = ptp.tile([128, IB], i32, tag="ue", name="ue", bufs=4)
                    nc.vector.tensor_scalar(
                        ue, ssc, EXP_A, EXP_B, alu.mult, alu.add
                    )
                    re = ptp.tile([128, IB], i32, tag="re", name="re", bufs=4)
                    nc.vector.tensor_scalar(
                        re, ue, EXP_MASK, EXP_OR, alu.bitwise_and, alu.bitwise_or
                    )
                    nc.vector._custom_dve(
                        exp_corr,
                        out=pt,
                        in0=re.bitcast(f32),
                        in1=ue.bitcast(f32),
                        s0=EXP_Q1,
                        s1=EXP_Q2,
                        imm2=EXP_Q0,
                    )
                else:
                    nc.scalar.activation(pt, ssc, EXP, scale=0.125)
                return pt

            def av(h, jt, pt):
                for ch in range(NCH):
                    nc.tensor.matmul(
                        o_augs[h][:, ch * 512 : (ch + 1) * 512],
                        lhsT=v_sb[:, jt, h, :],
                        rhs=pt[:, ch * 512 : (ch + 1) * 512],
                        start=(jt == 0),
                        stop=(jt == ST - 1),
                    )

            order = [(h, jt) for jt in range(ST) for h in heads]
            DEPTH = 6
            extra = list(extra or [])
            pts = {}
            for n, (h, jt) in enumerate(order):
                pts[(h, jt)] = scores(h, jt)
                if n >= DEPTH:
                    key = order[n - DEPTH]
                    av(*key, pts.pop(key))
                if extra and n % 3 == 2:
                    extra.pop(0)()
            for key in order[-DEPTH:]:
                av(*key, pts.pop(key))
            for fn in extra:
                fn()

            # normalize both heads. First copy o_aug to SBUF (releases the
            # PSUM slot for the next pair immediately), then broadcast the
            # colsum via PE outer product and multiply by its reciprocal.
            for h in heads:
                o_aug = o_augs[h]
                o_cp = norm.tile([65, IB], f32r, tag="ocp", name="o_cp")
                nc.vector.tensor_copy(o_cp, o_aug)
                cb_ps = ps.tile([64, IB], f32, tag="ps", name="cb_ps")
                for ch in range(NCH):
                    nc.tensor.matmul(
                        cb_ps[:, ch * 512 : (ch + 1) * 512],
                        lhsT=ones65[64:65, :],
                        rhs=o_cp[64:65, ch * 512 : (ch + 1) * 512],
                        start=True,
                        stop=True,
                    )
                rb_f = norm.tile([64, IB], f32, tag="rb_f", name="rb_f")
                nc.vector.reciprocal_approx_fast(rb_f, cb_ps)
                nc.vector.tensor_mul(o_sb[:, h, ib], o_cp[0:64, :], rb_f)

        def emit_out_block(ib, it):
            """Output projection for rows [ib*IB + it*128, +128)."""
            po = ps.tile([128, D], f32, tag="ps", name="po")
            for h in range(HPC):
                for ch in range(2):
                    nc.tensor.matmul(
                        po[:, ch * 512 : (ch + 1) * 512],
                        lhsT=o_sb[:, h, ib, it * 128 : (it + 1) * 128],
                        rhs=wo_r[:, h, ch * 512 : (ch + 1) * 512],
                        start=(h == 0),
                        stop=(h == HPC - 1),
                    )
            ot = outsb.tile([128, D], f32, tag="ot", name="ot")
            nc.vector.tensor_copy(ot, po)
            row = ib * IB + it * 128
            nc.sync.dma_start(out=outp[row : row + 128, :], in_=ot)

        # Partial output projection for i-block 1, heads 0-1 only, into an
        # SBUF accumulator (runs interleaved inside the last attention pair
        # so the tail only has heads 2-3 left).
        dacc = ctx.enter_context(tc.tile_pool(name="dacc", bufs=1))
        d1_acc = dacc.tile([128, 8, D], f32, name="d1_acc")

        def emit_d1_part01(it):
            po = ps.tile([128, D], f32, tag="ps", name="po")
            for h in range(2):
                for ch in range(2):
                    nc.tensor.matmul(
                        po[:, ch * 512 : (ch + 1) * 512],
                        lhsT=o_sb[:, h, 1, it * 128 : (it + 1) * 128],
                        rhs=wo_r[:, h, ch * 512 : (ch + 1) * 512],
                        start=(h == 0),
                        stop=(h == 1),
                    )
            nc.vector.tensor_copy(d1_acc[:, it], po)

        def emit_out_block2(it):
            """Tail: heads 2-3 of i-block 1 plus the accumulated 0-1 half."""
            po = ps.tile([128, D], f32, tag="ps", name="po")
            for h in range(2, HPC):
                for ch in range(2):
                    nc.tensor.matmul(
                        po[:, ch * 512 : (ch + 1) * 512],
                        lhsT=o_sb[:, h, 1, it * 128 : (it + 1) * 128],
                        rhs=wo_r[:, h, ch * 512 : (ch + 1) * 512],
                        start=(h == 2),
                        stop=(h == HPC - 1),
                    )
            ot = outsb.tile([128, D], f32, tag="ot", name="ot")
            nc.vector.tensor_add(ot, po, d1_acc[:, it])
            row = IB + it * 128
            nc.sync.dma_start(out=outp[row : row + 128, :], in_=ot)

        # Attention i-block 0 (two head-pairs), then i-block 1 with i-block
        # 0's output projection (and the first half of i-block 1's) inter-
        # leaved as PE filler, then the short tail.
        emit_head_pair(0, 0)
        emit_head_pair(0, 2)
        emit_head_pair(
            1, 0, extra=[lambda it=it: emit_out_block(0, it) for it in range(8)]
        )
        emit_head_pair(
            1, 2, extra=[lambda it=it: emit_d1_part01(it) for it in range(8)]
        )
        for it in range(8):
            emit_out_block2(it)


_PROGRAM = None


def _program():
    global _PROGRAM
    if _PROGRAM is None:
        nc = bacc.Bacc("TRN2", target_bir_lowering=False, debug=False)
        with tile.TileContext(nc) as tc:
            _emit(tc, nc)
        nc.compile()
        _PROGRAM = nc
    return _PROGRAM


def kernel(x, e, wq, wk, wv, wo, **_unused):
    x = np.asarray(x, dtype=np.float32)
    wq = np.asarray(wq, dtype=np.float32)
    wk = np.asarray(wk, dtype=np.float32)
    wv = np.asarray(wv, dtype=np.float32)
    wo = np.asarray(wo, dtype=np.float32)

    nc = _program()
    in_maps = []
    for c in range(NCORES):
        b, g = divmod(c, GROUPS)
        rows = slice(g * DLOC, (g + 1) * DLOC)
        in_maps.append(
            {
                "xT": np.ascontiguousarray(x[b].T),
                "wqT": np.ascontiguousarray(wq[rows, :].T),
                "wkT": np.ascontiguousarray(wk[rows, :].T),
                "wvT": np.ascontiguousarray(wv[rows, :].T),
                "woT": np.ascontiguousarray(wo[:, rows].T),
            }
        )

    res = run_bass_kernel_spmd(nc, in_maps, list(range(NCORES))).results
    out = np.empty((B, S, D), dtype=np.float32)
    for b in range(B):
        acc = res[b * GROUPS]["outp"].astype(np.float32)
        for g in range(1, GROUPS):
            acc = acc + res[b * GROUPS + g]["outp"]
        out[b] = acc
    return out
